# revision 1
# baseline (speedup 1.0000x reference)
"""Multi-head attention (Keras-style, relu-activated dense projections)
for Trainium2, SPMD across 8 NeuronCores.

Problem (full shapes):
    B, S, D, H = 4, 1024, 1024, 16 ; DH = 64
    qp = relu(q @ Wq + bq); kp = relu(k @ Wk + bk); vp = relu(v @ Wv + bv)
    per head h: scores = qh @ kh^T / 8 ; attn = softmax(scores)
    out = relu(concat_h(attn @ vh) @ Wo + bo)

Sharding: core c = (batch b = c//2, head-group g = c%2). Each core computes
the 8 heads of group g for batch b end-to-end and produces the partial
output projection  attn_out_g @ Wo[g*512:(g+1)*512, :]  (no bias / relu).
Host sums the two partials per batch, adds bo, applies relu.

Per-core dataflow (head pair hp = heads 2hp / 2hp+1):
  - host feeds q[b].T etc so projections contract d on the partition dim.
  - Q/K projections transposed: qpT/kpT [128, 4(hp), 1024(s)]; head 2hp at
    partitions 0:64, head 2hp+1 at 64:128 -> the K=64 score matmuls of a
    pair auto-land in different PE row groups and run concurrently.
  - scores pair writes one [128, 1024] 2-bank PSUM tile; one wide exp (ACT)
    emits ex [128, ut, 1024] bf16 (head A cols 0:512, B 512:1024).
  - attn@v: column-paired bf16 matmuls into nt[0:64] / nt[64:128].
  - softmax denominator: DVE tree-sums ex over ut, two K=128 matmuls with a
    ones column reduce partitions -> Z_A (psum row 0) / Z_B (row 32); a
    masked K=33 matmul broadcasts both to [128, 512]; wide DVE reciprocal +
    a single [128, 512] multiply writes attn_out.
  - output projection: full K=128 accumulating matmuls over head pairs.
  - matmuls in float32r (fp22, full PE rate) except the bf16 attention core.
"""

import numpy as np
from contextlib import ExitStack

import concourse.bass as bass
import concourse.mybir as mybir
import concourse.tile as tile
from concourse import bacc

# ---- constants (hardcoded per the contract; kernel.py must be self-contained)
B, S, D, H = 4, 1024, 1024, 16
DG = 512          # feature slice per core (8 heads)
HL = 8            # heads per core
DH = 64
P = 128
NCORES = 8
NJT = DG // P     # 4 feature tiles == head pairs
NST = S // P      # 8 sequence tiles
NDT = D // P      # 8 contraction tiles for projections
NPC = S // 512    # 2 query chunks of 512

F32 = mybir.dt.float32
F32R = mybir.dt.float32r
BF16 = mybir.dt.bfloat16
AF = mybir.ActivationFunctionType


def _d(ap):
    """View a float32 DRAM AP as float32r so DMAs into f32r tiles type-check.
    (walrus requires fp32r matmul operands to be *produced* as fp32r.)"""
    return ap.bitcast(F32R)


def build_bass():
    nc = bacc.Bacc("TRN2", target_bir_lowering=False, debug=False,
                   num_devices=NCORES)

    xqT = nc.dram_tensor("xqT", [D, S], F32, kind="ExternalInput").ap()
    xkT = nc.dram_tensor("xkT", [D, S], F32, kind="ExternalInput").ap()
    xvT = nc.dram_tensor("xvT", [D, S], F32, kind="ExternalInput").ap()
    wq = nc.dram_tensor("wq", [D, DG], F32, kind="ExternalInput").ap()
    wk = nc.dram_tensor("wk", [D, DG], F32, kind="ExternalInput").ap()
    wv = nc.dram_tensor("wv", [D, DG], F32, kind="ExternalInput").ap()
    bq = nc.dram_tensor("bq", [1, DG], F32, kind="ExternalInput").ap()
    bk = nc.dram_tensor("bk", [1, DG], F32, kind="ExternalInput").ap()
    bv = nc.dram_tensor("bv", [1, DG], F32, kind="ExternalInput").ap()
    wo = nc.dram_tensor("wo", [DG, D], F32, kind="ExternalInput").ap()
    ones_in = nc.dram_tensor("ones", [1, 512], F32, kind="ExternalInput").ap()
    bcm_in = nc.dram_tensor("bcmask", [33, P], F32, kind="ExternalInput").ap()
    out = nc.dram_tensor("out", [S, D], F32, kind="ExternalOutput").ap()

    with tile.TileContext(nc) as tc, ExitStack() as ctx, \
            nc.allow_low_precision(reason="fp32r/bf16 compute is intentional"):
        consts = ctx.enter_context(tc.tile_pool(name="consts", bufs=1))
        xpool = ctx.enter_context(tc.tile_pool(name="xpool", bufs=20))
        wpool = ctx.enter_context(tc.tile_pool(name="wpool", bufs=16))
        qkpool = ctx.enter_context(tc.tile_pool(name="qkpool", bufs=1))
        vpool = ctx.enter_context(tc.tile_pool(name="vpool", bufs=1))
        epool = ctx.enter_context(tc.tile_pool(name="epool", bufs=2))
        aopool = ctx.enter_context(tc.tile_pool(name="aopool", bufs=1))
        t1pool = ctx.enter_context(tc.tile_pool(name="t1pool", bufs=1))
        espool = ctx.enter_context(tc.tile_pool(name="espool", bufs=2))
        rpool = ctx.enter_context(tc.tile_pool(name="rpool", bufs=2))
        outpool = ctx.enter_context(tc.tile_pool(name="outpool", bufs=3))

        psA = ctx.enter_context(tc.tile_pool(name="psA", bufs=2, space="PSUM"))
        psB = ctx.enter_context(tc.tile_pool(name="psB", bufs=2, space="PSUM"))
        psZ = ctx.enter_context(tc.tile_pool(name="psZ", bufs=1, space="PSUM"))
        psD = ctx.enter_context(tc.tile_pool(name="psD", bufs=1, space="PSUM"))

        # --- constants
        ones = consts.tile([P, 512], F32R, tag="ones")
        nc.sync.dma_start(out=ones, in_=_d(ones_in.to_broadcast([P, 512])))
        onescol = consts.tile([P, 1], BF16, tag="onescol")
        nc.vector.memset(onescol, 1.0)
        bcmask = consts.tile([33, P], F32R, tag="bcmask")
        nc.sync.dma_start(out=bcmask, in_=_d(bcm_in))
        # zsb: persistent Z staging rows (0 and 32); fill once with finite
        # values so the masked K=33 broadcast matmul never reads NaNs.
        zsb = consts.tile([33, 512], F32R, tag="zsb")
        nc.sync.dma_start(out=zsb, in_=_d(ones_in.to_broadcast([33, 512])))

        bv_sb = consts.tile([1, DG], F32R, tag="bv")
        nc.sync.dma_start(out=bv_sb, in_=_d(bv))

        # --- transposed projections for Q and K
        qpT = qkpool.tile([P, NJT, S], F32R, tag="qpT")
        kpT = qkpool.tile([P, NJT, S], F32R, tag="kpT")

        # per-partition bias for the transposed projections (ACT bias input)
        bqT = consts.tile([P, NJT], F32, tag="bqT")
        nc.sync.dma_start(out=bqT, in_=bq[0, :].rearrange("(jt p) -> p jt", p=P))
        bkT = consts.tile([P, NJT], F32, tag="bkT")
        nc.sync.dma_start(out=bkT, in_=bk[0, :].rearrange("(jt p) -> p jt", p=P))

        def load_halves(xT, w):
            xmap = {}
            for pc in range(NPC):
                for dt_ in range(NDT):
                    xt = xpool.tile([P, 512], F32R, tag="xT")
                    nc.sync.dma_start(
                        out=xt,
                        in_=_d(xT[dt_ * P:(dt_ + 1) * P,
                                  pc * 512:(pc + 1) * 512]))
                    xmap[(dt_, pc)] = xt
            wts = []
            for dt_ in range(NDT):
                wt = wpool.tile([P, DG], F32R, tag="w")
                nc.sync.dma_start(out=wt, in_=_d(w[dt_ * P:(dt_ + 1) * P, :]))
                wts.append(wt)
            return xmap, wts

        for name, xT, w, bT, dst in (("q", xqT, wq, bqT, qpT),
                                     ("k", xkT, wk, bkT, kpT)):
            xmap, wts = load_halves(xT, w)
            for pc in range(NPC):
                for jt in range(NJT):
                    ps = psA.tile([P, 1024], F32, tag="ps")
                    half = ps[:, 0:512]
                    for dt_ in range(NDT):
                        nc.tensor.matmul(
                            half,
                            lhsT=wts[dt_][:, jt * P:(jt + 1) * P],
                            rhs=xmap[(dt_, pc)],
                            start=(dt_ == 0), stop=(dt_ == NDT - 1))
                    nc.scalar.activation(
                        dst[:, jt, pc * 512:(pc + 1) * 512], half, AF.Relu,
                        bias=bT[:, jt:jt + 1])

        # --- V projection, natural layout -> vpa [128, st, 512] bf16
        vpa = vpool.tile([P, NST, DG], BF16, tag="vpa")
        xmap, wts = load_halves(xvT, wv)
        for st in range(NST):
            ps = psA.tile([P, 1024], F32, tag="ps")
            half = ps[:, 0:512]
            for dt_ in range(NDT):
                nc.tensor.matmul(
                    half,
                    lhsT=xmap[(dt_, st // 4)][:, (st % 4) * P:(st % 4 + 1) * P],
                    rhs=wts[dt_],
                    start=(dt_ == 0), stop=False)
            nc.tensor.matmul(
                half, lhsT=ones[0:1, 0:P], rhs=bv_sb,
                start=False, stop=True)
            nc.scalar.activation(vpa[:, st, :], half, AF.Relu)

        # --- attention, one head pair x one 512-query chunk at a time.
        # pc outer: all head pairs of a query chunk finish together, so the
        # matching half of the output projection can start while the second
        # chunk's attention is still running.
        aoT3 = aopool.tile([P, NJT, S], F32R, tag="aoT3")

        # Wo by head pair (emitted here so its DMA runs during attention)
        wo3 = consts.tile([P, NJT, D], F32R, tag="wo3")
        for hp in range(NJT):
            nc.sync.dma_start(out=wo3[:, hp, :],
                              in_=_d(wo[hp * P:(hp + 1) * P, :]))

        for pc in range(NPC):
            pslice = slice(pc * 512, (pc + 1) * 512)
            for hp in range(NJT):
                hA, hB = 2 * hp, 2 * hp + 1
                ex = epool.tile([P, NST, 1024], BF16, tag="exp")
                for ut in range(NST):
                    uslice = slice(ut * P, (ut + 1) * P)
                    pw = psA.tile([P, 1024], F32, tag="ps")
                    nc.tensor.matmul(
                        pw[:, 0:512],
                        lhsT=kpT[0:DH, hp, uslice],
                        rhs=qpT[0:DH, hp, pslice],
                        start=True, stop=True)
                    nc.tensor.matmul(
                        pw[:, 512:1024],
                        lhsT=kpT[DH:P, hp, uslice],
                        rhs=qpT[DH:P, hp, pslice],
                        start=True, stop=True)
                    nc.scalar.activation(ex[:, ut, :], pw, AF.Exp, scale=0.125)
                # Z tree-sum over ut on DVE (overlaps the attn@v matmuls)
                t1 = t1pool.tile([P, 4, 1024], BF16, tag="t1")
                nc.vector.tensor_add(t1, ex[:, 0:4, :], ex[:, 4:8, :])
                nc.vector.tensor_add(t1[:, 0:2, :], t1[:, 0:2, :],
                                     t1[:, 2:4, :])
                exsum = espool.tile([P, 1024], BF16, tag="exsum")
                nc.vector.tensor_add(exsum, t1[:, 0, :], t1[:, 1, :])
                # Z_A -> psum row 0, Z_B -> psum row 32 (col group 1), then
                # stage into SBUF; emitted before attn@v so the copies are
                # long done when PE reaches the broadcast matmul.
                zps = psZ.tile([P, 512], F32, tag="z")
                nc.tensor.matmul(zps[0:1, :], lhsT=onescol,
                                 rhs=exsum[:, 0:512], start=True, stop=True)
                nc.tensor.matmul(zps[32:33, :], lhsT=onescol,
                                 rhs=exsum[:, 512:1024], start=True, stop=True)
                nc.vector.tensor_copy(zsb[0:1, :], zps[0:1, :])
                nc.vector.tensor_copy(zsb[32:33, :], zps[32:33, :])
                # attn @ v: column-paired accumulation over key tiles
                nt = psB.tile([P, 512], F32, tag="nt")
                for ut in range(NST):
                    nc.tensor.matmul(
                        nt[0:DH, :],
                        lhsT=vpa[:, ut, hA * DH:(hA + 1) * DH],
                        rhs=ex[:, ut, 0:512],
                        start=(ut == 0), stop=(ut == NST - 1),
                        skip_group_check=True)
                    nc.tensor.matmul(
                        nt[DH:P, :],
                        lhsT=vpa[:, ut, hB * DH:(hB + 1) * DH],
                        rhs=ex[:, ut, 512:1024],
                        start=(ut == 0), stop=(ut == NST - 1),
                        skip_group_check=True)
                # broadcast: rows 0:64 <- Z_A, rows 64:128 <- Z_B
                zbc = psZ.tile([P, 512], F32, tag="z")
                nc.tensor.matmul(zbc, lhsT=bcmask, rhs=zsb,
                                 start=True, stop=True)
                rcp = rpool.tile([P, 512], F32, tag="rcp")
                nc.vector.reciprocal_approx_fast(rcp, zbc)
                nc.vector.tensor_mul(aoT3[:, hp, pslice], nt, rcp)

            # output projection for this query chunk (pt = pc*4 .. pc*4+3)
            for pt in range(pc * 4, pc * 4 + 4):
                for jj in range(2):
                    po_ = psD.tile([P, 512], F32, tag="po")
                    for hp in range(NJT):
                        nc.tensor.matmul(
                            po_,
                            lhsT=aoT3[:, hp, pt * P:(pt + 1) * P],
                            rhs=wo3[:, hp, jj * 512:(jj + 1) * 512],
                            start=(hp == 0), stop=(hp == NJT - 1))
                    os_ = outpool.tile([P, 512], F32, tag="os")
                    nc.vector.tensor_copy(os_, po_)
                    nc.sync.dma_start(
                        out=out[pt * P:(pt + 1) * P, jj * 512:(jj + 1) * 512],
                        in_=os_)

    nc.compile()
    return nc


_CACHE = {}


def get_nc():
    if "nc" not in _CACHE:
        _CACHE["nc"] = build_bass()
    return _CACHE["nc"]


def make_bcmask():
    m = np.zeros((33, P), np.float32)
    m[0, 0:DH] = 1.0
    m[32, DH:P] = 1.0
    return m


def make_in_maps(q, k, v, Wq, bq, Wk, bk, Wv, bv, Wo, bo):
    q = np.asarray(q, np.float32)
    k = np.asarray(k, np.float32)
    v = np.asarray(v, np.float32)
    Wq = np.asarray(Wq, np.float32)
    Wk = np.asarray(Wk, np.float32)
    Wv = np.asarray(Wv, np.float32)
    Wo = np.asarray(Wo, np.float32)
    bq = np.asarray(bq, np.float32)
    bk = np.asarray(bk, np.float32)
    bv = np.asarray(bv, np.float32)

    qT = [np.ascontiguousarray(q[b].T) for b in range(B)]
    kT = [np.ascontiguousarray(k[b].T) for b in range(B)]
    vT = [np.ascontiguousarray(v[b].T) for b in range(B)]
    bcm = make_bcmask()

    in_maps = []
    for c in range(NCORES):
        b, g = divmod(c, 2)
        sl = slice(g * DG, (g + 1) * DG)
        in_maps.append({
            "xqT": qT[b],
            "xkT": kT[b],
            "xvT": vT[b],
            "wq": np.ascontiguousarray(Wq[:, sl]),
            "wk": np.ascontiguousarray(Wk[:, sl]),
            "wv": np.ascontiguousarray(Wv[:, sl]),
            "bq": np.ascontiguousarray(bq[sl]).reshape(1, DG),
            "bk": np.ascontiguousarray(bk[sl]).reshape(1, DG),
            "bv": np.ascontiguousarray(bv[sl]).reshape(1, DG),
            "wo": np.ascontiguousarray(Wo[sl, :]),
            "ones": np.ones((1, 512), np.float32),
            "bcmask": bcm,
        })
    return in_maps


def combine_outputs(parts, bo):
    bo = np.asarray(bo, np.float32)
    out = np.empty((B, S, D), np.float32)
    for b in range(B):
        out[b] = np.maximum(parts[2 * b] + parts[2 * b + 1] + bo[None, :], 0.0)
    return out


def run(in_maps, trace=False, **kwargs):
    from concourse.bass_utils import run_bass_kernel_spmd
    nc = get_nc()
    return run_bass_kernel_spmd(nc, in_maps, list(range(NCORES)),
                                trace=trace, **kwargs)


def kernel(q, k, v, Wq, bq, Wk, bk, Wv, bv, Wo, bo):
    in_maps = make_in_maps(q, k, v, Wq, bq, Wk, bk, Wv, bv, Wo, bo)
    res = run(in_maps)
    parts = [res.results[c]["out"] for c in range(NCORES)]
    return combine_outputs(parts, bo)



# revision 3
# speedup vs baseline: 1.3272x; 1.3272x over previous
"""Multi-head attention (Keras-style, relu-activated dense projections)
for Trainium2, SPMD across 8 NeuronCores.

Problem (full shapes):
    B, S, D, H = 4, 1024, 1024, 16 ; DH = 64
    qp = relu(q @ Wq + bq); kp = relu(k @ Wk + bk); vp = relu(v @ Wv + bv)
    per head h: scores = qh @ kh^T / 8 ; attn = softmax(scores)
    out = relu(concat_h(attn @ vh) @ Wo + bo)

Sharding: core c = (batch b = c//2, head-group g = c%2). Each core computes
the 8 heads of group g for batch b end-to-end and produces the partial
output projection  attn_out_g @ Wo[g*512:(g+1)*512, :]  (no bias / relu).
Host sums the two partials per batch, adds bo, applies relu.

v2 design notes (vs the fp32r baseline):
  - all matmul operands bf16 (host-cast): halves input DMA, enables FWL
    weight loads, avoids fp32_mode=HIGH matmuls.
  - DMA issue order matches consumption order (w/x interleaved per dt),
    K then Q(pc0) first so the first score matmuls start ~15us in.
  - attention emitted block-by-block (scores+exp first, then prev block's
    attn@v / Z / normalize) so the Tile scheduler keeps ACT saturated with
    exp while PE fills stalls with attn@v / V-projection / out-projection
    work. PE stays dense -> HAM clock gate stays warm.
  - Z staging: single [33,512] PSUM->SBUF copy (garbage rows masked by the
    broadcast matmul), relu+bias on DVE via dual-op tensor_scalar.
  - partial outputs returned bf16; host combines in fp32.
"""

import numpy as np
import ml_dtypes
from contextlib import ExitStack

import concourse.bass as bass
import concourse.mybir as mybir
import concourse.tile as tile
from concourse import bacc

# ---- constants (hardcoded per the contract; kernel.py must be self-contained)
B, S, D, H = 4, 1024, 1024, 16
DG = 512          # feature slice per core (8 heads)
HL = 8            # heads per core
DH = 64
P = 128
NCORES = 8
NJT = DG // P     # 4 feature tiles == head pairs
NST = S // P      # 8 sequence tiles
NDT = D // P      # 8 contraction tiles for projections
NPC = S // 512    # 2 query chunks of 512

F32 = mybir.dt.float32
BF16 = mybir.dt.bfloat16
AF = mybir.ActivationFunctionType
ALU = mybir.AluOpType
NPBF16 = ml_dtypes.bfloat16


def build_bass():
    nc = bacc.Bacc("TRN2", target_bir_lowering=False, debug=False,
                   num_devices=NCORES)

    xqT = nc.dram_tensor("xqT", [D, S], BF16, kind="ExternalInput").ap()
    xkT = nc.dram_tensor("xkT", [D, S], BF16, kind="ExternalInput").ap()
    xvT = nc.dram_tensor("xvT", [D, S], BF16, kind="ExternalInput").ap()
    wq = nc.dram_tensor("wq", [D, DG], BF16, kind="ExternalInput").ap()
    wk = nc.dram_tensor("wk", [D, DG], BF16, kind="ExternalInput").ap()
    wv = nc.dram_tensor("wv", [D, DG], BF16, kind="ExternalInput").ap()
    bq = nc.dram_tensor("bq", [1, DG], F32, kind="ExternalInput").ap()
    bk = nc.dram_tensor("bk", [1, DG], F32, kind="ExternalInput").ap()
    bv = nc.dram_tensor("bv", [1, DG], BF16, kind="ExternalInput").ap()
    wo = nc.dram_tensor("wo", [DG, D], BF16, kind="ExternalInput").ap()
    out = nc.dram_tensor("out", [S, D], BF16, kind="ExternalOutput").ap()

    with tile.TileContext(nc) as tc, ExitStack() as ctx, \
            nc.allow_low_precision(reason="bf16 compute is intentional"):
        consts = ctx.enter_context(tc.tile_pool(name="consts", bufs=1))
        xpool = ctx.enter_context(tc.tile_pool(name="xpool", bufs=36))
        wpool = ctx.enter_context(tc.tile_pool(name="wpool", bufs=24))
        qkpool = ctx.enter_context(tc.tile_pool(name="qkpool", bufs=1))
        vpool = ctx.enter_context(tc.tile_pool(name="vpool", bufs=1))
        epool = ctx.enter_context(tc.tile_pool(name="epool", bufs=2))
        aopool = ctx.enter_context(tc.tile_pool(name="aopool", bufs=1))
        t1pool = ctx.enter_context(tc.tile_pool(name="t1pool", bufs=2))
        espool = ctx.enter_context(tc.tile_pool(name="espool", bufs=2))
        rpool = ctx.enter_context(tc.tile_pool(name="rpool", bufs=2))
        zsbpool = ctx.enter_context(tc.tile_pool(name="zsbpool", bufs=2))
        outpool = ctx.enter_context(tc.tile_pool(name="outpool", bufs=4))

        # PSUM: psA 2x[128,1024] (scores + QK proj chains) = 4 banks,
        # psB 2x[128,512] (attn@v accum + V proj chains) = 2 banks,
        # psZD 2x[128,512] (Z staging/broadcast + out-proj chains) = 2 banks.
        psA = ctx.enter_context(tc.tile_pool(name="psA", bufs=2, space="PSUM"))
        psB = ctx.enter_context(tc.tile_pool(name="psB", bufs=2, space="PSUM"))
        psZD = ctx.enter_context(tc.tile_pool(name="psZD", bufs=2,
                                              space="PSUM"))

        # --- constants (memset where possible; tiny DMAs otherwise)
        onescol = consts.tile([P, 1], BF16, tag="onescol")
        nc.vector.memset(onescol, 1.0)
        onesrow = consts.tile([1, P], BF16, tag="onesrow")
        nc.vector.memset(onesrow, 1.0)
        bcmask = consts.tile([33, P], BF16, tag="bcmask")
        nc.vector.memset(bcmask, 0.0)
        nc.vector.memset(bcmask[0:1, 0:DH], 1.0)
        nc.vector.memset(bcmask[32:33, DH:P], 1.0)

        bv_sb = consts.tile([1, DG], BF16, tag="bv")
        nc.sync.dma_start(out=bv_sb, in_=bv)
        bqT = consts.tile([P, NJT], F32, tag="bqT")
        nc.sync.dma_start(out=bqT, in_=bq[0, :].rearrange("(jt p) -> p jt", p=P))
        bkT = consts.tile([P, NJT], F32, tag="bkT")
        nc.sync.dma_start(out=bkT, in_=bk[0, :].rearrange("(jt p) -> p jt", p=P))

        # dummy exp to pull the ACT table load off the critical path
        dummy = consts.tile([1, 8], BF16, tag="dummy")
        nc.scalar.activation(dummy, bcmask[0:1, 0:8], AF.Exp)

        # --- input DMAs, in consumption order.
        # Phase 1 feeds K proj (pc0+pc1) and Q proj (pc0): interleave
        # wk/xk/wq/xq(pc0) per dt so the projection chains chase the DMAs.
        xk_t, xq_t, xv_t = {}, {}, {}
        wk_t, wq_t, wv_t = [None] * NDT, [None] * NDT, [None] * NDT

        def dma_x(xmap, xT, dt_, pc):
            t = xpool.tile([P, 512], BF16, tag="xT")
            nc.sync.dma_start(
                out=t, in_=xT[dt_ * P:(dt_ + 1) * P, pc * 512:(pc + 1) * 512])
            xmap[(dt_, pc)] = t

        def dma_w(wlist, w, dt_):
            t = wpool.tile([P, DG], BF16, tag="w")
            nc.sync.dma_start(out=t, in_=w[dt_ * P:(dt_ + 1) * P, :])
            wlist[dt_] = t

        for dt_ in range(NDT):
            dma_w(wk_t, wk, dt_)
            dma_x(xk_t, xkT, dt_, 0)
            dma_w(wq_t, wq, dt_)
            dma_x(xq_t, xqT, dt_, 0)
        for dt_ in range(NDT):
            dma_x(xk_t, xkT, dt_, 1)
        for dt_ in range(NDT):
            dma_x(xq_t, xqT, dt_, 1)
        for dt_ in range(NDT):
            dma_w(wv_t, wv, dt_)
            dma_x(xv_t, xvT, dt_, 0)
            dma_x(xv_t, xvT, dt_, 1)
        wo3 = consts.tile([P, NJT, D], BF16, tag="wo3")
        for hp in range(NJT):
            nc.sync.dma_start(out=wo3[:, hp, :],
                              in_=wo[hp * P:(hp + 1) * P, :])

        # --- transposed Q/K projections: dst[:, jt, pc*512:] = relu(w.T@x + b)
        qpT = qkpool.tile([P, NJT, S], BF16, tag="qpT")
        kpT = qkpool.tile([P, NJT, S], BF16, tag="kpT")

        def qk_wave(wts, xmap, pc, dst, bT):
            # one wave = 4 chains (jt 0..3) on 2 psA tiles (2 halves each)
            tiles = [psA.tile([P, 1024], F32, tag="ps", name=f"ps_wave{n}")
                     for n in range(2)]
            for dt_ in range(NDT):
                for jt in range(NJT):
                    half = tiles[jt // 2][:, (jt % 2) * 512:(jt % 2 + 1) * 512]
                    nc.tensor.matmul(
                        half,
                        lhsT=wts[dt_][:, jt * P:(jt + 1) * P],
                        rhs=xmap[(dt_, pc)],
                        start=(dt_ == 0), stop=(dt_ == NDT - 1))
            for jt in range(NJT):
                half = tiles[jt // 2][:, (jt % 2) * 512:(jt % 2 + 1) * 512]
                nc.vector.tensor_scalar(
                    out=dst[:, jt, pc * 512:(pc + 1) * 512], in0=half,
                    scalar1=bT[:, jt:jt + 1], scalar2=0.0,
                    op0=ALU.add, op1=ALU.max)

        # order: K(pc0), Q(pc0) -> first scores asap; then K(pc1), Q(pc1)
        qk_wave(wk_t, xk_t, 0, kpT, bkT)
        qk_wave(wq_t, xq_t, 0, qpT, bqT)
        qk_wave(wk_t, xk_t, 1, kpT, bkT)
        qk_wave(wq_t, xq_t, 1, qpT, bqT)

        # --- V projection, natural layout -> vpa [128, st, 512] bf16
        # chains on psB tiles (1 bank each); emitted before attention so the
        # scheduler uses them as PE filler during early exp ticks.
        vpa = vpool.tile([P, NST, DG], BF16, tag="vpa")
        for st in range(NST):
            ps = psB.tile([P, 512], F32, tag="nt")
            for dt_ in range(NDT):
                nc.tensor.matmul(
                    ps,
                    lhsT=xv_t[(dt_, st // 4)][:, (st % 4) * P:(st % 4 + 1) * P],
                    rhs=wv_t[dt_],
                    start=(dt_ == 0), stop=False)
            nc.tensor.matmul(ps, lhsT=onesrow, rhs=bv_sb,
                             start=False, stop=True)
            nc.vector.tensor_scalar(out=vpa[:, st, :], in0=ps,
                                    scalar1=0.0, scalar2=None, op0=ALU.max)

        # --- attention, software-pipelined across 8 (pc, hp) blocks.
        aoT3 = aopool.tile([P, NJT, S], BF16, tag="aoT3")
        blocks = [(pc, hp) for pc in range(NPC) for hp in range(NJT)]
        ex_tiles = {}

        def emit_scores(i):
            pc, hp = blocks[i]
            pslice = slice(pc * 512, (pc + 1) * 512)
            ex = epool.tile([P, NST, 1024], BF16, tag="exp")
            ex_tiles[i] = ex
            for ut in range(NST):
                uslice = slice(ut * P, (ut + 1) * P)
                pw = psA.tile([P, 1024], F32, tag="ps")
                nc.tensor.matmul(
                    pw[:, 0:512],
                    lhsT=kpT[0:DH, hp, uslice],
                    rhs=qpT[0:DH, hp, pslice],
                    start=True, stop=True)
                nc.tensor.matmul(
                    pw[:, 512:1024],
                    lhsT=kpT[DH:P, hp, uslice],
                    rhs=qpT[DH:P, hp, pslice],
                    start=True, stop=True)
                nc.scalar.activation(ex[:, ut, :], pw, AF.Exp, scale=0.125)

        def emit_attn_finish(i):
            pc, hp = blocks[i]
            pslice = slice(pc * 512, (pc + 1) * 512)
            hA, hB = 2 * hp, 2 * hp + 1
            ex = ex_tiles.pop(i)
            # attn @ v: column-paired accumulation over key tiles
            nt = psB.tile([P, 512], F32, tag="nt")
            for ut in range(NST):
                nc.tensor.matmul(
                    nt[0:DH, :],
                    lhsT=vpa[:, ut, hA * DH:(hA + 1) * DH],
                    rhs=ex[:, ut, 0:512],
                    start=(ut == 0), stop=(ut == NST - 1),
                    skip_group_check=True)
                nc.tensor.matmul(
                    nt[DH:P, :],
                    lhsT=vpa[:, ut, hB * DH:(hB + 1) * DH],
                    rhs=ex[:, ut, 512:1024],
                    start=(ut == 0), stop=(ut == NST - 1),
                    skip_group_check=True)
            # softmax denominator: DVE tree-sum over ut, two K=128 matmuls
            # with a ones column reduce partitions -> Z_A (row 0), Z_B (row
            # 32); one [33,512] copy stages both (garbage rows masked later).
            t1 = t1pool.tile([P, 4, 1024], BF16, tag="t1")
            nc.vector.tensor_add(t1, ex[:, 0:4, :], ex[:, 4:8, :])
            nc.vector.tensor_add(t1[:, 0:2, :], t1[:, 0:2, :], t1[:, 2:4, :])
            exsum = espool.tile([P, 1024], BF16, tag="exsum")
            nc.vector.tensor_add(exsum, t1[:, 0, :], t1[:, 1, :])
            zps = psZD.tile([P, 512], F32, tag="po")
            nc.tensor.matmul(zps[0:1, :], lhsT=onescol,
                             rhs=exsum[:, 0:512], start=True, stop=True)
            nc.tensor.matmul(zps[32:33, :], lhsT=onescol,
                             rhs=exsum[:, 512:1024], start=True, stop=True)
            zsb = zsbpool.tile([33, 512], BF16, tag="zsb")
            nc.vector.tensor_copy(zsb, zps[0:33, :])
            # broadcast: rows 0:64 <- Z_A, rows 64:128 <- Z_B
            zbc = psZD.tile([P, 512], F32, tag="po")
            nc.tensor.matmul(zbc, lhsT=bcmask, rhs=zsb,
                             start=True, stop=True)
            rcp = rpool.tile([P, 512], F32, tag="rcp")
            nc.vector.reciprocal_approx_fast(rcp, zbc)
            nc.vector.tensor_mul(aoT3[:, hp, pslice], nt, rcp)

        def emit_outproj(pc):
            for pt in range(pc * 4, pc * 4 + 4):
                for jj in range(2):
                    po_ = psZD.tile([P, 512], F32, tag="po")
                    for hp in range(NJT):
                        nc.tensor.matmul(
                            po_,
                            lhsT=aoT3[:, hp, pt * P:(pt + 1) * P],
                            rhs=wo3[:, hp, jj * 512:(jj + 1) * 512],
                            start=(hp == 0), stop=(hp == NJT - 1))
                    os_ = outpool.tile([P, 512], BF16, tag="os")
                    nc.vector.tensor_copy(os_, po_)
                    nc.sync.dma_start(
                        out=out[pt * P:(pt + 1) * P, jj * 512:(jj + 1) * 512],
                        in_=os_)

        for i in range(len(blocks)):
            emit_scores(i)
            if i > 0:
                emit_attn_finish(i - 1)
            if i - 1 == 3:
                emit_outproj(0)
        emit_attn_finish(len(blocks) - 1)
        emit_outproj(1)

    nc.compile()
    return nc


_CACHE = {}


def get_nc():
    if "nc" not in _CACHE:
        _CACHE["nc"] = build_bass()
    return _CACHE["nc"]


def make_in_maps(q, k, v, Wq, bq, Wk, bk, Wv, bv, Wo, bo):
    q = np.asarray(q, np.float32)
    k = np.asarray(k, np.float32)
    v = np.asarray(v, np.float32)
    Wq = np.asarray(Wq, np.float32)
    Wk = np.asarray(Wk, np.float32)
    Wv = np.asarray(Wv, np.float32)
    Wo = np.asarray(Wo, np.float32)
    bq = np.asarray(bq, np.float32)
    bk = np.asarray(bk, np.float32)
    bv = np.asarray(bv, np.float32)

    qT = [np.ascontiguousarray(q[b].T).astype(NPBF16) for b in range(B)]
    kT = [np.ascontiguousarray(k[b].T).astype(NPBF16) for b in range(B)]
    vT = [np.ascontiguousarray(v[b].T).astype(NPBF16) for b in range(B)]

    in_maps = []
    for c in range(NCORES):
        b, g = divmod(c, 2)
        sl = slice(g * DG, (g + 1) * DG)
        in_maps.append({
            "xqT": qT[b],
            "xkT": kT[b],
            "xvT": vT[b],
            "wq": np.ascontiguousarray(Wq[:, sl]).astype(NPBF16),
            "wk": np.ascontiguousarray(Wk[:, sl]).astype(NPBF16),
            "wv": np.ascontiguousarray(Wv[:, sl]).astype(NPBF16),
            "bq": np.ascontiguousarray(bq[sl]).reshape(1, DG),
            "bk": np.ascontiguousarray(bk[sl]).reshape(1, DG),
            "bv": np.ascontiguousarray(bv[sl]).reshape(1, DG).astype(NPBF16),
            "wo": np.ascontiguousarray(Wo[sl, :]).astype(NPBF16),
        })
    return in_maps


def combine_outputs(parts, bo):
    bo = np.asarray(bo, np.float32)
    out = np.empty((B, S, D), np.float32)
    for b in range(B):
        p0 = np.asarray(parts[2 * b], np.float32)
        p1 = np.asarray(parts[2 * b + 1], np.float32)
        out[b] = np.maximum(p0 + p1 + bo[None, :], 0.0)
    return out


def run(in_maps, trace=False, **kwargs):
    from concourse.bass_utils import run_bass_kernel_spmd
    nc = get_nc()
    return run_bass_kernel_spmd(nc, in_maps, list(range(NCORES)),
                                trace=trace, **kwargs)


def kernel(q, k, v, Wq, bq, Wk, bk, Wv, bv, Wo, bo):
    in_maps = make_in_maps(q, k, v, Wq, bq, Wk, bk, Wv, bv, Wo, bo)
    res = run(in_maps)
    parts = [res.results[c]["out"] for c in range(NCORES)]
    return combine_outputs(parts, bo)


# revision 4
# speedup vs baseline: 1.5473x; 1.1658x over previous
"""Multi-head attention (Keras-style, relu-activated dense projections)
for Trainium2, SPMD across 8 NeuronCores.

Problem (full shapes):
    B, S, D, H = 4, 1024, 1024, 16 ; DH = 64
    qp = relu(q @ Wq + bq); kp = relu(k @ Wk + bk); vp = relu(v @ Wv + bv)
    per head h: scores = qh @ kh^T / 8 ; attn = softmax(scores)
    out = relu(concat_h(attn @ vh) @ Wo + bo)

Sharding: core c = (batch b = c//2, head-group g = c%2). Each core computes
the 8 heads of group g for batch b end-to-end and produces the partial
output projection  attn_out_g @ Wo[g*512:(g+1)*512, :]  (no bias / relu).
Host sums the two partials per batch, adds bo, applies relu.

v3 design notes:
  - all matmul operands bf16 (host-cast): halves input DMA, FWL weight
    loads, no fp32_mode=HIGH matmuls.
  - inputs land via ~30 large chunked DMAs (512KB) instead of ~95 small
    ones: each dma_start costs ~600ns of serialized HWDGE issue, which
    paced the whole projection era in v2.
  - attention software-pipelined across 8 (pc,hp) blocks: scores+exp of
    block i emitted before attn@v/Z/normalize of block i-1, V projection
    and out-projection emitted as PE filler; Tile's readiness scheduler
    keeps ACT saturated with exp and PE dense (HAM stays warm).
  - engine balance: exp + QK bias-relu + Z staging copy on ACT; tree-sum,
    V relu, reciprocal, normalize, out copies on DVE.
  - partial outputs returned bf16; host combines in fp32.
"""

import numpy as np
import ml_dtypes
from contextlib import ExitStack

import concourse.bass as bass
import concourse.mybir as mybir
import concourse.tile as tile
from concourse import bacc

# ---- constants (hardcoded per the contract; kernel.py must be self-contained)
B, S, D, H = 4, 1024, 1024, 16
DG = 512          # feature slice per core (8 heads)
HL = 8            # heads per core
DH = 64
P = 128
NCORES = 8
NJT = DG // P     # 4 feature tiles == head pairs
NST = S // P      # 8 sequence tiles
NDT = D // P      # 8 contraction tiles for projections
NPC = S // 512    # 2 query chunks of 512

F32 = mybir.dt.float32
BF16 = mybir.dt.bfloat16
AF = mybir.ActivationFunctionType
ALU = mybir.AluOpType
NPBF16 = ml_dtypes.bfloat16


def build_bass():
    nc = bacc.Bacc("TRN2", target_bir_lowering=False, debug=False,
                   num_devices=NCORES)

    xqT = nc.dram_tensor("xqT", [D, S], BF16, kind="ExternalInput").ap()
    xkT = nc.dram_tensor("xkT", [D, S], BF16, kind="ExternalInput").ap()
    xvT = nc.dram_tensor("xvT", [D, S], BF16, kind="ExternalInput").ap()
    wq = nc.dram_tensor("wq", [D, DG], BF16, kind="ExternalInput").ap()
    wk = nc.dram_tensor("wk", [D, DG], BF16, kind="ExternalInput").ap()
    wv = nc.dram_tensor("wv", [D, DG], BF16, kind="ExternalInput").ap()
    bq = nc.dram_tensor("bq", [1, DG], F32, kind="ExternalInput").ap()
    bk = nc.dram_tensor("bk", [1, DG], F32, kind="ExternalInput").ap()
    bv = nc.dram_tensor("bv", [1, DG], BF16, kind="ExternalInput").ap()
    wo = nc.dram_tensor("wo", [DG, D], BF16, kind="ExternalInput").ap()
    out = nc.dram_tensor("out", [S, D], BF16, kind="ExternalOutput").ap()

    with tile.TileContext(nc) as tc, ExitStack() as ctx, \
            nc.allow_low_precision(reason="bf16 compute is intentional"):
        consts = ctx.enter_context(tc.tile_pool(name="consts", bufs=1))
        xpool = ctx.enter_context(tc.tile_pool(name="xpool", bufs=1))
        qkpool = ctx.enter_context(tc.tile_pool(name="qkpool", bufs=1))
        vpool = ctx.enter_context(tc.tile_pool(name="vpool", bufs=1))
        epool = ctx.enter_context(tc.tile_pool(name="epool", bufs=2))
        aopool = ctx.enter_context(tc.tile_pool(name="aopool", bufs=1))
        t1pool = ctx.enter_context(tc.tile_pool(name="t1pool", bufs=1))
        espool = ctx.enter_context(tc.tile_pool(name="espool", bufs=2))
        rpool = ctx.enter_context(tc.tile_pool(name="rpool", bufs=2))
        zsbpool = ctx.enter_context(tc.tile_pool(name="zsbpool", bufs=2))
        outpool = ctx.enter_context(tc.tile_pool(name="outpool", bufs=2))

        # PSUM: psA 2x[128,1024] (scores + QK proj chains) = 4 banks,
        # psB 2x[128,512] (attn@v accum + V proj chains) = 2 banks,
        # psZD 2x[128,512] (Z staging/broadcast + out-proj chains) = 2 banks.
        psA = ctx.enter_context(tc.tile_pool(name="psA", bufs=2, space="PSUM"))
        psB = ctx.enter_context(tc.tile_pool(name="psB", bufs=2, space="PSUM"))
        psZD = ctx.enter_context(tc.tile_pool(name="psZD", bufs=2,
                                              space="PSUM"))

        # --- constants (memset where possible; tiny DMAs otherwise)
        onescol = consts.tile([P, 1], BF16, tag="onescol")
        nc.vector.memset(onescol, 1.0)
        onesrow = consts.tile([1, P], BF16, tag="onesrow")
        nc.vector.memset(onesrow, 1.0)
        bcmask = consts.tile([33, P], BF16, tag="bcmask")
        nc.vector.memset(bcmask, 0.0)
        nc.vector.memset(bcmask[0:1, 0:DH], 1.0)
        nc.vector.memset(bcmask[32:33, DH:P], 1.0)

        bv_sb = consts.tile([1, DG], BF16, tag="bv")
        nc.sync.dma_start(out=bv_sb, in_=bv)
        bqT = consts.tile([P, NJT], F32, tag="bqT")
        nc.sync.dma_start(out=bqT, in_=bq[0, :].rearrange("(jt p) -> p jt", p=P))
        bkT = consts.tile([P, NJT], F32, tag="bkT")
        nc.sync.dma_start(out=bkT, in_=bk[0, :].rearrange("(jt p) -> p jt", p=P))

        # dummy exp to pull the ACT table load off the critical path
        dummy = consts.tile([1, 8], BF16, tag="dummy")
        nc.scalar.activation(dummy, bcmask[0:1, 0:8], AF.Exp)

        # --- inputs: big SBUF tiles, few large DMAs, consumption order.
        xk_a = xpool.tile([P, NDT, S], BF16, tag="xk")
        xq_a = xpool.tile([P, NDT, S], BF16, tag="xq")
        xv_a = xpool.tile([P, NDT, S], BF16, tag="xv")
        wk_a = xpool.tile([P, NDT, DG], BF16, tag="wk")
        wq_a = xpool.tile([P, NDT, DG], BF16, tag="wq")
        wv_a = xpool.tile([P, NDT, DG], BF16, tag="wv")
        wo3 = consts.tile([P, NJT, D], BF16, tag="wo3")

        def dma_w(dst, w):
            for c in range(2):
                nc.sync.dma_start(
                    out=dst[:, 4 * c:4 * c + 4, :],
                    in_=w[c * 512:(c + 1) * 512, :].rearrange(
                        "(f p) g -> p f g", p=P))

        def dma_x(dst, xT):
            for c in range(4):
                nc.sync.dma_start(
                    out=dst[:, 2 * c:2 * c + 2, :],
                    in_=xT[c * 256:(c + 1) * 256, :].rearrange(
                        "(f p) s -> p f s", p=P))

        dma_w(wk_a, wk)
        dma_x(xk_a, xkT)
        dma_w(wq_a, wq)
        dma_x(xq_a, xqT)
        dma_w(wv_a, wv)
        dma_x(xv_a, xvT)
        for c in range(2):
            nc.sync.dma_start(
                out=wo3[:, 2 * c:2 * c + 2, :],
                in_=wo[c * 256:(c + 1) * 256, :].rearrange(
                    "(f p) d2 -> p f d2", p=P))

        # --- transposed Q/K projections: dst[:, jt, pc*512:] = relu(w.T@x+b)
        qpT = qkpool.tile([P, NJT, S], BF16, tag="qpT")
        kpT = qkpool.tile([P, NJT, S], BF16, tag="kpT")

        def qk_wave(w_a, x_a, pc, dst, bT):
            # one wave = 4 chains (jt 0..3) on 2 psA tiles (2 halves each)
            tiles = [psA.tile([P, 1024], F32, tag="ps", name=f"ps_wave{n}")
                     for n in range(2)]
            for dt_ in range(NDT):
                for jt in range(NJT):
                    half = tiles[jt // 2][:, (jt % 2) * 512:(jt % 2 + 1) * 512]
                    nc.tensor.matmul(
                        half,
                        lhsT=w_a[:, dt_, jt * P:(jt + 1) * P],
                        rhs=x_a[:, dt_, pc * 512:(pc + 1) * 512],
                        start=(dt_ == 0), stop=(dt_ == NDT - 1))
            for jt in range(NJT):
                half = tiles[jt // 2][:, (jt % 2) * 512:(jt % 2 + 1) * 512]
                nc.scalar.activation(
                    dst[:, jt, pc * 512:(pc + 1) * 512], half, AF.Relu,
                    bias=bT[:, jt:jt + 1])

        # order: K(pc0), Q(pc0) -> first scores asap; then K(pc1), Q(pc1)
        qk_wave(wk_a, xk_a, 0, kpT, bkT)
        qk_wave(wq_a, xq_a, 0, qpT, bqT)
        qk_wave(wk_a, xk_a, 1, kpT, bkT)
        qk_wave(wq_a, xq_a, 1, qpT, bqT)

        # --- V projection, natural layout -> vpa [128, st, 512] bf16
        # chains on psB tiles (1 bank each); emitted before attention so the
        # scheduler uses them as PE filler during early exp ticks.
        vpa = vpool.tile([P, NST, DG], BF16, tag="vpa")
        for st in range(NST):
            ps = psB.tile([P, 512], F32, tag="nt")
            for dt_ in range(NDT):
                nc.tensor.matmul(
                    ps,
                    lhsT=xv_a[:, dt_, st * P:(st + 1) * P],
                    rhs=wv_a[:, dt_, :],
                    start=(dt_ == 0), stop=False)
            nc.tensor.matmul(ps, lhsT=onesrow, rhs=bv_sb,
                             start=False, stop=True)
            nc.vector.tensor_scalar(out=vpa[:, st, :], in0=ps,
                                    scalar1=0.0, scalar2=None, op0=ALU.max)

        # --- attention, software-pipelined across 8 (pc, hp) blocks.
        aoT3 = aopool.tile([P, NJT, S], BF16, tag="aoT3")
        blocks = [(pc, hp) for pc in range(NPC) for hp in range(NJT)]
        ex_tiles = {}

        def emit_scores(i):
            pc, hp = blocks[i]
            pslice = slice(pc * 512, (pc + 1) * 512)
            ex = epool.tile([P, NST, 1024], BF16, tag="exp")
            ex_tiles[i] = ex
            for ut in range(NST):
                uslice = slice(ut * P, (ut + 1) * P)
                pw = psA.tile([P, 1024], F32, tag="ps")
                nc.tensor.matmul(
                    pw[:, 0:512],
                    lhsT=kpT[0:DH, hp, uslice],
                    rhs=qpT[0:DH, hp, pslice],
                    start=True, stop=True)
                nc.tensor.matmul(
                    pw[:, 512:1024],
                    lhsT=kpT[DH:P, hp, uslice],
                    rhs=qpT[DH:P, hp, pslice],
                    start=True, stop=True)
                nc.scalar.activation(ex[:, ut, :], pw, AF.Exp, scale=0.125)

        def emit_attn_finish(i):
            pc, hp = blocks[i]
            pslice = slice(pc * 512, (pc + 1) * 512)
            hA, hB = 2 * hp, 2 * hp + 1
            ex = ex_tiles.pop(i)
            # attn @ v: column-paired accumulation over key tiles
            nt = psB.tile([P, 512], F32, tag="nt")
            for ut in range(NST):
                nc.tensor.matmul(
                    nt[0:DH, :],
                    lhsT=vpa[:, ut, hA * DH:(hA + 1) * DH],
                    rhs=ex[:, ut, 0:512],
                    start=(ut == 0), stop=(ut == NST - 1),
                    skip_group_check=True)
                nc.tensor.matmul(
                    nt[DH:P, :],
                    lhsT=vpa[:, ut, hB * DH:(hB + 1) * DH],
                    rhs=ex[:, ut, 512:1024],
                    start=(ut == 0), stop=(ut == NST - 1),
                    skip_group_check=True)
            # softmax denominator: DVE tree-sum over ut, two K=128 matmuls
            # with a ones column reduce partitions -> Z_A (row 0), Z_B (row
            # 32); one [33,512] ACT copy stages both (garbage rows masked by
            # the broadcast matmul).
            t1 = t1pool.tile([P, 4, 1024], BF16, tag="t1")
            nc.vector.tensor_add(t1, ex[:, 0:4, :], ex[:, 4:8, :])
            nc.vector.tensor_add(t1[:, 0:2, :], t1[:, 0:2, :], t1[:, 2:4, :])
            exsum = espool.tile([P, 1024], BF16, tag="exsum")
            nc.vector.tensor_add(exsum, t1[:, 0, :], t1[:, 1, :])
            zps = psZD.tile([P, 512], F32, tag="po")
            nc.tensor.matmul(zps[0:1, :], lhsT=onescol,
                             rhs=exsum[:, 0:512], start=True, stop=True)
            nc.tensor.matmul(zps[32:33, :], lhsT=onescol,
                             rhs=exsum[:, 512:1024], start=True, stop=True)
            zsb = zsbpool.tile([33, 512], BF16, tag="zsb")
            nc.scalar.copy(zsb, zps[0:33, :])
            # broadcast: rows 0:64 <- Z_A, rows 64:128 <- Z_B
            zbc = psZD.tile([P, 512], F32, tag="po")
            nc.tensor.matmul(zbc, lhsT=bcmask, rhs=zsb,
                             start=True, stop=True)
            rcp = rpool.tile([P, 512], F32, tag="rcp")
            nc.vector.reciprocal_approx_fast(rcp, zbc)
            nc.vector.tensor_mul(aoT3[:, hp, pslice], nt, rcp)

        def emit_outproj(pc):
            for pt in range(pc * 4, pc * 4 + 4):
                os_ = outpool.tile([P, 1024], BF16, tag="os")
                for jj in range(2):
                    po_ = psZD.tile([P, 512], F32, tag="po")
                    for hp in range(NJT):
                        nc.tensor.matmul(
                            po_,
                            lhsT=aoT3[:, hp, pt * P:(pt + 1) * P],
                            rhs=wo3[:, hp, jj * 512:(jj + 1) * 512],
                            start=(hp == 0), stop=(hp == NJT - 1))
                    nc.vector.tensor_copy(os_[:, jj * 512:(jj + 1) * 512], po_)
                nc.sync.dma_start(out=out[pt * P:(pt + 1) * P, :], in_=os_)

        for i in range(len(blocks)):
            emit_scores(i)
            if i > 0:
                emit_attn_finish(i - 1)
            if i - 1 == 3:
                emit_outproj(0)
        emit_attn_finish(len(blocks) - 1)
        emit_outproj(1)

    nc.compile()
    return nc


_CACHE = {}


def get_nc():
    if "nc" not in _CACHE:
        _CACHE["nc"] = build_bass()
    return _CACHE["nc"]


def make_in_maps(q, k, v, Wq, bq, Wk, bk, Wv, bv, Wo, bo):
    q = np.asarray(q, np.float32)
    k = np.asarray(k, np.float32)
    v = np.asarray(v, np.float32)
    Wq = np.asarray(Wq, np.float32)
    Wk = np.asarray(Wk, np.float32)
    Wv = np.asarray(Wv, np.float32)
    Wo = np.asarray(Wo, np.float32)
    bq = np.asarray(bq, np.float32)
    bk = np.asarray(bk, np.float32)
    bv = np.asarray(bv, np.float32)

    qT = [np.ascontiguousarray(q[b].T).astype(NPBF16) for b in range(B)]
    kT = [np.ascontiguousarray(k[b].T).astype(NPBF16) for b in range(B)]
    vT = [np.ascontiguousarray(v[b].T).astype(NPBF16) for b in range(B)]

    in_maps = []
    for c in range(NCORES):
        b, g = divmod(c, 2)
        sl = slice(g * DG, (g + 1) * DG)
        in_maps.append({
            "xqT": qT[b],
            "xkT": kT[b],
            "xvT": vT[b],
            "wq": np.ascontiguousarray(Wq[:, sl]).astype(NPBF16),
            "wk": np.ascontiguousarray(Wk[:, sl]).astype(NPBF16),
            "wv": np.ascontiguousarray(Wv[:, sl]).astype(NPBF16),
            "bq": np.ascontiguousarray(bq[sl]).reshape(1, DG),
            "bk": np.ascontiguousarray(bk[sl]).reshape(1, DG),
            "bv": np.ascontiguousarray(bv[sl]).reshape(1, DG).astype(NPBF16),
            "wo": np.ascontiguousarray(Wo[sl, :]).astype(NPBF16),
        })
    return in_maps


def combine_outputs(parts, bo):
    bo = np.asarray(bo, np.float32)
    out = np.empty((B, S, D), np.float32)
    for b in range(B):
        p0 = np.asarray(parts[2 * b], np.float32)
        p1 = np.asarray(parts[2 * b + 1], np.float32)
        out[b] = np.maximum(p0 + p1 + bo[None, :], 0.0)
    return out


def run(in_maps, trace=False, **kwargs):
    from concourse.bass_utils import run_bass_kernel_spmd
    nc = get_nc()
    return run_bass_kernel_spmd(nc, in_maps, list(range(NCORES)),
                                trace=trace, **kwargs)


def kernel(q, k, v, Wq, bq, Wk, bk, Wv, bv, Wo, bo):
    in_maps = make_in_maps(q, k, v, Wq, bq, Wk, bk, Wv, bv, Wo, bo)
    res = run(in_maps)
    parts = [res.results[c]["out"] for c in range(NCORES)]
    return combine_outputs(parts, bo)


# revision 8
# speedup vs baseline: 1.7306x; 1.1185x over previous
"""Multi-head attention (Keras-style, relu-activated dense projections)
for Trainium2, SPMD across 8 NeuronCores.

Problem (full shapes):
    B, S, D, H = 4, 1024, 1024, 16 ; DH = 64
    qp = relu(q @ Wq + bq); kp = relu(k @ Wk + bk); vp = relu(v @ Wv + bv)
    per head h: scores = qh @ kh^T / 8 ; attn = softmax(scores)
    out = relu(concat_h(attn @ vh) @ Wo + bo)

Sharding: core c = (batch b = c//2, head-group g = c%2). Each core computes
the 8 heads of group g for batch b end-to-end and produces the partial
output projection  attn_out_g @ Wo[g*512:(g+1)*512, :]  (no bias / relu).
Host sums the two partials per batch, adds bo, applies relu.

v3 design notes:
  - all matmul operands bf16 (host-cast): halves input DMA, FWL weight
    loads, no fp32_mode=HIGH matmuls.
  - inputs land via ~30 large chunked DMAs (512KB) instead of ~95 small
    ones: each dma_start costs ~600ns of serialized HWDGE issue, which
    paced the whole projection era in v2.
  - attention software-pipelined across 8 (pc,hp) blocks: scores+exp of
    block i emitted before attn@v/Z/normalize of block i-1, V projection
    and out-projection emitted as PE filler; Tile's readiness scheduler
    keeps ACT saturated with exp and PE dense (HAM stays warm).
  - engine balance: exp + QK bias-relu + Z staging copy on ACT; tree-sum,
    V relu, reciprocal, normalize, out copies on DVE.
  - partial outputs returned bf16; host combines in fp32.
"""

import numpy as np
import ml_dtypes
from contextlib import ExitStack

import concourse.bass as bass
import concourse.mybir as mybir
import concourse.tile as tile
from concourse import bacc

# ---- constants (hardcoded per the contract; kernel.py must be self-contained)
B, S, D, H = 4, 1024, 1024, 16
DG = 512          # feature slice per core (8 heads)
HL = 8            # heads per core
DH = 64
P = 128
NCORES = 8
NJT = DG // P     # 4 feature tiles == head pairs
NST = S // P      # 8 sequence tiles
NDT = D // P      # 8 contraction tiles for projections
NPC = S // 512    # 2 query chunks of 512

F32 = mybir.dt.float32
BF16 = mybir.dt.bfloat16
AF = mybir.ActivationFunctionType
ALU = mybir.AluOpType
NPBF16 = ml_dtypes.bfloat16


def build_bass():
    nc = bacc.Bacc("TRN2", target_bir_lowering=False, debug=False,
                   num_devices=NCORES)

    xqT = nc.dram_tensor("xqT", [D, S], BF16, kind="ExternalInput").ap()
    xkT = nc.dram_tensor("xkT", [D, S], BF16, kind="ExternalInput").ap()
    xvT = nc.dram_tensor("xvT", [D, S], BF16, kind="ExternalInput").ap()
    wq = nc.dram_tensor("wq", [D, DG], BF16, kind="ExternalInput").ap()
    wk = nc.dram_tensor("wk", [D, DG], BF16, kind="ExternalInput").ap()
    wv = nc.dram_tensor("wv", [D, DG], BF16, kind="ExternalInput").ap()
    bq = nc.dram_tensor("bq", [1, DG], F32, kind="ExternalInput").ap()
    bk = nc.dram_tensor("bk", [1, DG], F32, kind="ExternalInput").ap()
    bv = nc.dram_tensor("bv", [1, DG], BF16, kind="ExternalInput").ap()
    wo = nc.dram_tensor("wo", [DG, D], BF16, kind="ExternalInput").ap()
    out = nc.dram_tensor("out", [S, D], BF16, kind="ExternalOutput").ap()

    with tile.TileContext(nc) as tc, ExitStack() as ctx, \
            nc.allow_low_precision(reason="bf16 compute is intentional"):
        consts = ctx.enter_context(tc.tile_pool(name="consts", bufs=1))
        xpool = ctx.enter_context(tc.tile_pool(name="xpool", bufs=1))
        qkpool = ctx.enter_context(tc.tile_pool(name="qkpool", bufs=1))
        vpool = ctx.enter_context(tc.tile_pool(name="vpool", bufs=1))
        epool = ctx.enter_context(tc.tile_pool(name="epool", bufs=3))
        aopool = ctx.enter_context(tc.tile_pool(name="aopool", bufs=1))
        t1pool = ctx.enter_context(tc.tile_pool(name="t1pool", bufs=1))
        espool = ctx.enter_context(tc.tile_pool(name="espool", bufs=2))
        rpool = ctx.enter_context(tc.tile_pool(name="rpool", bufs=2))
        zsbpool = ctx.enter_context(tc.tile_pool(name="zsbpool", bufs=2))
        outpool = ctx.enter_context(tc.tile_pool(name="outpool", bufs=2))

        # PSUM: psA 2x[128,1024] (scores + QK proj chains) = 4 banks,
        # psB 2x[128,512] (attn@v accum + V proj chains) = 2 banks,
        # psZD 2x[128,512] (Z staging/broadcast + out-proj chains) = 2 banks.
        psA = ctx.enter_context(tc.tile_pool(name="psA", bufs=2, space="PSUM"))
        psB = ctx.enter_context(tc.tile_pool(name="psB", bufs=2, space="PSUM"))
        psZD = ctx.enter_context(tc.tile_pool(name="psZD", bufs=2,
                                              space="PSUM"))

        # --- constants (memset where possible; tiny DMAs otherwise)
        onescol = consts.tile([P, 1], BF16, tag="onescol")
        nc.vector.memset(onescol, 1.0)
        onesrow = consts.tile([1, P], BF16, tag="onesrow")
        nc.vector.memset(onesrow, 1.0)
        bcmask = consts.tile([33, P], BF16, tag="bcmask")
        nc.vector.memset(bcmask, 0.0)
        nc.vector.memset(bcmask[0:1, 0:DH], 1.0)
        nc.vector.memset(bcmask[32:33, DH:P], 1.0)

        bv_sb = consts.tile([1, DG], BF16, tag="bv")
        nc.sync.dma_start(out=bv_sb, in_=bv)
        bqT = consts.tile([P, NJT], F32, tag="bqT")
        nc.sync.dma_start(out=bqT, in_=bq[0, :].rearrange("(jt p) -> p jt", p=P))
        bkT = consts.tile([P, NJT], F32, tag="bkT")
        nc.sync.dma_start(out=bkT, in_=bk[0, :].rearrange("(jt p) -> p jt", p=P))

        # dummy exp to pull the ACT table load off the critical path
        dummy = consts.tile([1, 8], BF16, tag="dummy")
        nc.scalar.activation(dummy, bcmask[0:1, 0:8], AF.Exp)

        # --- inputs: big SBUF tiles, large DMAs in criticality order.
        # HWDGE rings fair-share bandwidth across in-flight DMAs, so the
        # critical set (K + Q jt0 inputs -> first scores block) is issued
        # first and everything else after.
        xk_a = xpool.tile([P, NDT, S], BF16, tag="xk")
        xq_a = xpool.tile([P, NDT, S], BF16, tag="xq")
        xv_a = xpool.tile([P, NDT, S], BF16, tag="xv")
        wk_a = xpool.tile([P, NDT, DG], BF16, tag="wk")
        wq_a = xpool.tile([P, NDT, DG], BF16, tag="wq")
        wv_a = xpool.tile([P, NDT, DG], BF16, tag="wv")
        wo3 = consts.tile([P, NJT, D], BF16, tag="wo3")

        def dma_w_jt(dst, w, jt):
            nc.sync.dma_start(
                out=dst[:, :, jt * P:(jt + 1) * P],
                in_=w[:, jt * P:(jt + 1) * P].rearrange(
                    "(f p) g -> p f g", p=P))

        def dma_x(dst, xT):
            for c in range(4):
                nc.sync.dma_start(
                    out=dst[:, 2 * c:2 * c + 2, :],
                    in_=xT[c * 256:(c + 1) * 256, :].rearrange(
                        "(f p) s -> p f s", p=P))

        dma_w_jt(wk_a, wk, 0)
        dma_x(xk_a, xkT)
        dma_w_jt(wq_a, wq, 0)
        dma_x(xq_a, xqT)
        for jt in range(1, NJT):
            dma_w_jt(wk_a, wk, jt)
            dma_w_jt(wq_a, wq, jt)
        for c in range(2):
            nc.sync.dma_start(
                out=wv_a[:, 4 * c:4 * c + 4, :],
                in_=wv[c * 512:(c + 1) * 512, :].rearrange(
                    "(f p) g -> p f g", p=P))
        dma_x(xv_a, xvT)
        for c in range(2):
            nc.sync.dma_start(
                out=wo3[:, 2 * c:2 * c + 2, :],
                in_=wo[c * 256:(c + 1) * 256, :].rearrange(
                    "(f p) d2 -> p f d2", p=P))

        # --- transposed Q/K projections: dst[:, jt, pc*512:] = relu(w.T@x+b)
        # per-jt waves: one K tile (pc0|pc1 halves) + one Q tile, so the
        # hp=jt0 scores block is ready after the first wave.
        qpT = qkpool.tile([P, NJT, S], BF16, tag="qpT")
        kpT = qkpool.tile([P, NJT, S], BF16, tag="kpT")

        def qk_wave(jt):
            tk = psA.tile([P, 1024], F32, tag="ps", name=f"ps_k{jt}")
            tq = psA.tile([P, 1024], F32, tag="ps", name=f"ps_q{jt}")
            for dt_ in range(NDT):
                for pc in range(NPC):
                    nc.tensor.matmul(
                        tk[:, pc * 512:(pc + 1) * 512],
                        lhsT=wk_a[:, dt_, jt * P:(jt + 1) * P],
                        rhs=xk_a[:, dt_, pc * 512:(pc + 1) * 512],
                        start=(dt_ == 0), stop=(dt_ == NDT - 1))
                    nc.tensor.matmul(
                        tq[:, pc * 512:(pc + 1) * 512],
                        lhsT=wq_a[:, dt_, jt * P:(jt + 1) * P],
                        rhs=xq_a[:, dt_, pc * 512:(pc + 1) * 512],
                        start=(dt_ == 0), stop=(dt_ == NDT - 1))
            for pc in range(NPC):
                nc.scalar.activation(
                    kpT[:, jt, pc * 512:(pc + 1) * 512],
                    tk[:, pc * 512:(pc + 1) * 512], AF.Relu,
                    bias=bkT[:, jt:jt + 1])
                nc.scalar.activation(
                    qpT[:, jt, pc * 512:(pc + 1) * 512],
                    tq[:, pc * 512:(pc + 1) * 512], AF.Relu,
                    bias=bqT[:, jt:jt + 1])

        for jt in range(NJT):
            qk_wave(jt)

        # --- V projection, natural layout -> vpa [128, st, 512] bf16
        # chains on psB tiles (1 bank each); emitted after the first two
        # scores blocks so it fills PE gaps without starving the exp stream.
        vpa = vpool.tile([P, NST, DG], BF16, tag="vpa")

        def emit_vproj():
            for st in range(NST):
                ps = psB.tile([P, 512], F32, tag="nt")
                for dt_ in range(NDT):
                    nc.tensor.matmul(
                        ps,
                        lhsT=xv_a[:, dt_, st * P:(st + 1) * P],
                        rhs=wv_a[:, dt_, :],
                        start=(dt_ == 0), stop=False)
                nc.tensor.matmul(ps, lhsT=onesrow, rhs=bv_sb,
                                 start=False, stop=True)
                nc.vector.tensor_scalar(out=vpa[:, st, :], in0=ps,
                                        scalar1=0.0, scalar2=None, op0=ALU.max)

        # --- attention, software-pipelined across 8 (pc, hp) blocks.
        aoT3 = aopool.tile([P, NJT, S], BF16, tag="aoT3")
        blocks = [(pc, hp) for pc in range(NPC) for hp in range(NJT)]
        ex_tiles = {}

        def emit_scores(i):
            pc, hp = blocks[i]
            pslice = slice(pc * 512, (pc + 1) * 512)
            ex = epool.tile([P, NST, 1024], BF16, tag="exp")
            ex_tiles[i] = ex
            for ut in range(NST):
                uslice = slice(ut * P, (ut + 1) * P)
                pw = psA.tile([P, 1024], F32, tag="ps")
                nc.tensor.matmul(
                    pw[:, 0:512],
                    lhsT=kpT[0:DH, hp, uslice],
                    rhs=qpT[0:DH, hp, pslice],
                    start=True, stop=True)
                nc.tensor.matmul(
                    pw[:, 512:1024],
                    lhsT=kpT[DH:P, hp, uslice],
                    rhs=qpT[DH:P, hp, pslice],
                    start=True, stop=True)
                nc.scalar.activation(ex[:, ut, :], pw, AF.Exp, scale=0.125)

        def emit_attn_finish(i):
            pc, hp = blocks[i]
            pslice = slice(pc * 512, (pc + 1) * 512)
            hA, hB = 2 * hp, 2 * hp + 1
            ex = ex_tiles.pop(i)
            # attn @ v: column-paired accumulation over key tiles
            nt = psB.tile([P, 512], F32, tag="nt")
            for ut in range(NST):
                nc.tensor.matmul(
                    nt[0:DH, :],
                    lhsT=vpa[:, ut, hA * DH:(hA + 1) * DH],
                    rhs=ex[:, ut, 0:512],
                    start=(ut == 0), stop=(ut == NST - 1),
                    skip_group_check=True)
                nc.tensor.matmul(
                    nt[DH:P, :],
                    lhsT=vpa[:, ut, hB * DH:(hB + 1) * DH],
                    rhs=ex[:, ut, 512:1024],
                    start=(ut == 0), stop=(ut == NST - 1),
                    skip_group_check=True)
            # softmax denominator: DVE tree-sum over ut, two K=128 matmuls
            # with a ones column reduce partitions -> Z_A (row 0), Z_B (row
            # 32); one [33,512] ACT copy stages both (garbage rows masked by
            # the broadcast matmul).
            t1 = t1pool.tile([P, 4, 1024], BF16, tag="t1")
            nc.vector.tensor_add(t1, ex[:, 0:4, :], ex[:, 4:8, :])
            nc.vector.tensor_add(t1[:, 0:2, :], t1[:, 0:2, :], t1[:, 2:4, :])
            exsum = espool.tile([P, 1024], BF16, tag="exsum")
            nc.vector.tensor_add(exsum, t1[:, 0, :], t1[:, 1, :])
            zps = psZD.tile([P, 512], F32, tag="po")
            nc.tensor.matmul(zps[0:1, :], lhsT=onescol,
                             rhs=exsum[:, 0:512], start=True, stop=True)
            nc.tensor.matmul(zps[32:33, :], lhsT=onescol,
                             rhs=exsum[:, 512:1024], start=True, stop=True)
            zsb = zsbpool.tile([33, 512], BF16, tag="zsb")
            nc.scalar.copy(zsb, zps[0:33, :])
            # broadcast: rows 0:64 <- Z_A, rows 64:128 <- Z_B
            zbc = psZD.tile([P, 512], F32, tag="po")
            nc.tensor.matmul(zbc, lhsT=bcmask, rhs=zsb,
                             start=True, stop=True)
            rcp = rpool.tile([P, 512], F32, tag="rcp")
            nc.vector.reciprocal_approx_fast(rcp, zbc)
            nc.vector.tensor_mul(aoT3[:, hp, pslice], nt, rcp)

        def emit_outproj(pc):
            for pt in range(pc * 4, pc * 4 + 4):
                os_ = outpool.tile([P, 1024], BF16, tag="os")
                for jj in range(2):
                    po_ = psZD.tile([P, 512], F32, tag="po")
                    for hp in range(NJT):
                        nc.tensor.matmul(
                            po_,
                            lhsT=aoT3[:, hp, pt * P:(pt + 1) * P],
                            rhs=wo3[:, hp, jj * 512:(jj + 1) * 512],
                            start=(hp == 0), stop=(hp == NJT - 1))
                    nc.vector.tensor_copy(os_[:, jj * 512:(jj + 1) * 512], po_)
                nc.sync.dma_start(out=out[pt * P:(pt + 1) * P, :], in_=os_)

        # scores run two blocks ahead of attn@v (ex triple-buffered); the V
        # projection is emitted after the first two scores blocks so the exp
        # stream outranks it on PE.
        for i in range(len(blocks)):
            emit_scores(i)
            if i == 1:
                emit_vproj()
            if i >= 2:
                emit_attn_finish(i - 2)
            if i - 2 == 3:
                emit_outproj(0)
        emit_attn_finish(len(blocks) - 2)
        emit_attn_finish(len(blocks) - 1)
        emit_outproj(1)

    nc.compile()
    return nc


_CACHE = {}


def get_nc():
    if "nc" not in _CACHE:
        _CACHE["nc"] = build_bass()
    return _CACHE["nc"]


def make_in_maps(q, k, v, Wq, bq, Wk, bk, Wv, bv, Wo, bo):
    q = np.asarray(q, np.float32)
    k = np.asarray(k, np.float32)
    v = np.asarray(v, np.float32)
    Wq = np.asarray(Wq, np.float32)
    Wk = np.asarray(Wk, np.float32)
    Wv = np.asarray(Wv, np.float32)
    Wo = np.asarray(Wo, np.float32)
    bq = np.asarray(bq, np.float32)
    bk = np.asarray(bk, np.float32)
    bv = np.asarray(bv, np.float32)

    qT = [np.ascontiguousarray(q[b].T).astype(NPBF16) for b in range(B)]
    kT = [np.ascontiguousarray(k[b].T).astype(NPBF16) for b in range(B)]
    vT = [np.ascontiguousarray(v[b].T).astype(NPBF16) for b in range(B)]

    in_maps = []
    for c in range(NCORES):
        b, g = divmod(c, 2)
        sl = slice(g * DG, (g + 1) * DG)
        in_maps.append({
            "xqT": qT[b],
            "xkT": kT[b],
            "xvT": vT[b],
            "wq": np.ascontiguousarray(Wq[:, sl]).astype(NPBF16),
            "wk": np.ascontiguousarray(Wk[:, sl]).astype(NPBF16),
            "wv": np.ascontiguousarray(Wv[:, sl]).astype(NPBF16),
            "bq": np.ascontiguousarray(bq[sl]).reshape(1, DG),
            "bk": np.ascontiguousarray(bk[sl]).reshape(1, DG),
            "bv": np.ascontiguousarray(bv[sl]).reshape(1, DG).astype(NPBF16),
            "wo": np.ascontiguousarray(Wo[sl, :]).astype(NPBF16),
        })
    return in_maps


def combine_outputs(parts, bo):
    bo = np.asarray(bo, np.float32)
    out = np.empty((B, S, D), np.float32)
    for b in range(B):
        p0 = np.asarray(parts[2 * b], np.float32)
        p1 = np.asarray(parts[2 * b + 1], np.float32)
        out[b] = np.maximum(p0 + p1 + bo[None, :], 0.0)
    return out


def run(in_maps, trace=False, **kwargs):
    from concourse.bass_utils import run_bass_kernel_spmd
    nc = get_nc()
    return run_bass_kernel_spmd(nc, in_maps, list(range(NCORES)),
                                trace=trace, **kwargs)


def kernel(q, k, v, Wq, bq, Wk, bk, Wv, bv, Wo, bo):
    in_maps = make_in_maps(q, k, v, Wq, bq, Wk, bk, Wv, bv, Wo, bo)
    res = run(in_maps)
    parts = [res.results[c]["out"] for c in range(NCORES)]
    return combine_outputs(parts, bo)


# revision 14
# speedup vs baseline: 1.8983x; 1.0969x over previous
"""Multi-head attention (Keras-style, relu-activated dense projections)
for Trainium2, SPMD across 8 NeuronCores.

Problem (full shapes):
    B, S, D, H = 4, 1024, 1024, 16 ; DH = 64
    qp = relu(q @ Wq + bq); kp = relu(k @ Wk + bk); vp = relu(v @ Wv + bv)
    per head h: scores = qh @ kh^T / 8 ; attn = softmax(scores)
    out = relu(concat_h(attn @ vh) @ Wo + bo)

Sharding: core c = (batch b = c//2, head-group g = c%2). Each core computes
the 8 heads of group g for batch b end-to-end and produces the partial
output projection  attn_out_g @ Wo[g*512:(g+1)*512, :]  (no bias / relu).
Host sums the two partials per batch, adds bo, applies relu.

v3 design notes:
  - all matmul operands bf16 (host-cast): halves input DMA, FWL weight
    loads, no fp32_mode=HIGH matmuls.
  - inputs land via ~30 large chunked DMAs (512KB) instead of ~95 small
    ones: each dma_start costs ~600ns of serialized HWDGE issue, which
    paced the whole projection era in v2.
  - attention software-pipelined across 8 (pc,hp) blocks: scores+exp of
    block i emitted before attn@v/Z/normalize of block i-1, V projection
    and out-projection emitted as PE filler; Tile's readiness scheduler
    keeps ACT saturated with exp and PE dense (HAM stays warm).
  - engine balance: exp + QK bias-relu + Z staging copy on ACT; tree-sum,
    V relu, reciprocal, normalize, out copies on DVE.
  - partial outputs returned bf16; host combines in fp32.
"""

import numpy as np
import ml_dtypes
from contextlib import ExitStack

import concourse.bass as bass
import concourse.mybir as mybir
import concourse.tile as tile
from concourse import bacc

# ---- constants (hardcoded per the contract; kernel.py must be self-contained)
B, S, D, H = 4, 1024, 1024, 16
DG = 512          # feature slice per core (8 heads)
HL = 8            # heads per core
DH = 64
P = 128
NCORES = 8
NJT = DG // P     # 4 feature tiles == head pairs
NST = S // P      # 8 sequence tiles
NDT = D // P      # 8 contraction tiles for projections
NPC = S // 512    # 2 query chunks of 512

F32 = mybir.dt.float32
BF16 = mybir.dt.bfloat16
FP8 = mybir.dt.float8e4
AF = mybir.ActivationFunctionType
ALU = mybir.AluOpType
DR = mybir.MatmulPerfMode.DoubleRow
NPBF16 = ml_dtypes.bfloat16
NPFP8 = ml_dtypes.float8_e4m3


def build_bass():
    nc = bacc.Bacc("TRN2", target_bir_lowering=False, debug=False,
                   num_devices=NCORES)

    xqT = nc.dram_tensor("xqT", [D, S], FP8, kind="ExternalInput").ap()
    xkT = nc.dram_tensor("xkT", [D, S], FP8, kind="ExternalInput").ap()
    xvT = nc.dram_tensor("xvT", [D, S], FP8, kind="ExternalInput").ap()
    wq = nc.dram_tensor("wq", [D, DG], FP8, kind="ExternalInput").ap()
    wk = nc.dram_tensor("wk", [D, DG], FP8, kind="ExternalInput").ap()
    wv = nc.dram_tensor("wv", [D, DG], FP8, kind="ExternalInput").ap()
    bq = nc.dram_tensor("bq", [1, DG], F32, kind="ExternalInput").ap()
    bk = nc.dram_tensor("bk", [1, DG], F32, kind="ExternalInput").ap()
    bv = nc.dram_tensor("bv", [1, DG], BF16, kind="ExternalInput").ap()
    wo = nc.dram_tensor("wo", [DG, D], BF16, kind="ExternalInput").ap()
    out = nc.dram_tensor("out", [S, D], BF16, kind="ExternalOutput").ap()

    with tile.TileContext(nc) as tc, ExitStack() as ctx, \
            nc.allow_low_precision(reason="bf16 compute is intentional"):
        consts = ctx.enter_context(tc.tile_pool(name="consts", bufs=1))
        xpool = ctx.enter_context(tc.tile_pool(name="xpool", bufs=1))
        qkpool = ctx.enter_context(tc.tile_pool(name="qkpool", bufs=1))
        vpool = ctx.enter_context(tc.tile_pool(name="vpool", bufs=1))
        epool = ctx.enter_context(tc.tile_pool(name="epool", bufs=3))
        aopool = ctx.enter_context(tc.tile_pool(name="aopool", bufs=1))
        t1pool = ctx.enter_context(tc.tile_pool(name="t1pool", bufs=1))
        espool = ctx.enter_context(tc.tile_pool(name="espool", bufs=2))
        rpool = ctx.enter_context(tc.tile_pool(name="rpool", bufs=2))
        zsbpool = ctx.enter_context(tc.tile_pool(name="zsbpool", bufs=2))
        outpool = ctx.enter_context(tc.tile_pool(name="outpool", bufs=2))

        # PSUM: psA 2x[128,1024] (scores + QK proj chains) = 4 banks,
        # psB 2x[128,512] (attn@v accum + V proj chains) = 2 banks,
        # psZD 2x[128,512] (Z staging/broadcast + out-proj chains) = 2 banks.
        psA = ctx.enter_context(tc.tile_pool(name="psA", bufs=2, space="PSUM"))
        psB = ctx.enter_context(tc.tile_pool(name="psB", bufs=2, space="PSUM"))
        psZD = ctx.enter_context(tc.tile_pool(name="psZD", bufs=2,
                                              space="PSUM"))

        # --- constants (memset where possible; tiny DMAs otherwise)
        onescol = consts.tile([P, 1], BF16, tag="onescol")
        nc.vector.memset(onescol, 1.0)
        onesrow = consts.tile([1, P], BF16, tag="onesrow")
        nc.vector.memset(onesrow, 1.0)
        bcmask = consts.tile([33, P], BF16, tag="bcmask")
        nc.vector.memset(bcmask, 0.0)
        nc.vector.memset(bcmask[0:1, 0:DH], 1.0)
        nc.vector.memset(bcmask[32:33, DH:P], 1.0)

        bv_sb = consts.tile([1, DG], BF16, tag="bv")
        nc.sync.dma_start(out=bv_sb, in_=bv)
        bqT = consts.tile([P, NJT], F32, tag="bqT")
        nc.sync.dma_start(out=bqT, in_=bq[0, :].rearrange("(jt p) -> p jt", p=P))
        bkT = consts.tile([P, NJT], F32, tag="bkT")
        nc.sync.dma_start(out=bkT, in_=bk[0, :].rearrange("(jt p) -> p jt", p=P))

        # dummy exp to pull the ACT table load off the critical path
        dummy = consts.tile([1, 8], BF16, tag="dummy")
        nc.scalar.activation(dummy, bcmask[0:1, 0:8], AF.Exp)

        # --- inputs: big SBUF tiles, large DMAs in criticality order.
        # HWDGE rings fair-share bandwidth across in-flight DMAs, so the
        # critical set (K + Q jt0 inputs -> first scores block) is issued
        # first and everything else after.
        xk_a = xpool.tile([P, NDT, S], FP8, tag="xk")
        xq_a = xpool.tile([P, NDT, S], FP8, tag="xq")
        xv_a = xpool.tile([P, NDT, S], FP8, tag="xv")
        wk_a = xpool.tile([P, NDT, DG], FP8, tag="wk")
        wq_a = xpool.tile([P, NDT, DG], FP8, tag="wq")
        wv_a = xpool.tile([P, NDT, DG], FP8, tag="wv")
        wo3 = consts.tile([P, NJT, D], BF16, tag="wo3")

        def dma_w_jt(dst, w, jt):
            nc.sync.dma_start(
                out=dst[:, :, jt * P:(jt + 1) * P],
                in_=w[:, jt * P:(jt + 1) * P].rearrange(
                    "(f p) g -> p f g", p=P))

        def dma_x(dst, xT):
            for c in range(4):
                nc.sync.dma_start(
                    out=dst[:, 2 * c:2 * c + 2, :],
                    in_=xT[c * 256:(c + 1) * 256, :].rearrange(
                        "(f p) s -> p f s", p=P))

        dma_w_jt(wk_a, wk, 0)
        dma_x(xk_a, xkT)
        dma_w_jt(wq_a, wq, 0)
        dma_x(xq_a, xqT)
        for jt in range(1, NJT):
            dma_w_jt(wk_a, wk, jt)
            dma_w_jt(wq_a, wq, jt)
        for c in range(2):
            nc.sync.dma_start(
                out=wv_a[:, 4 * c:4 * c + 4, :],
                in_=wv[c * 512:(c + 1) * 512, :].rearrange(
                    "(f p) g -> p f g", p=P))
        dma_x(xv_a, xvT)
        for c in range(2):
            nc.sync.dma_start(
                out=wo3[:, 2 * c:2 * c + 2, :],
                in_=wo[c * 256:(c + 1) * 256, :].rearrange(
                    "(f p) d2 -> p f d2", p=P))

        # --- transposed Q/K projections: dst[:, jt, pc*512:] = relu(w.T@x+b)
        # per-jt waves: one K tile (pc0|pc1 halves) + one Q tile, so the
        # hp=jt0 scores block is ready after the first wave.
        qpT = qkpool.tile([P, NJT, S], BF16, tag="qpT")
        kpT = qkpool.tile([P, NJT, S], BF16, tag="kpT")

        def qk_wave(jt):
            # fp8 DoubleRow: each matmul contracts a dt-pair (K=256 virtual)
            tk = psA.tile([P, 1024], F32, tag="ps", name=f"ps_k{jt}")
            tq = psA.tile([P, 1024], F32, tag="ps", name=f"ps_q{jt}")
            for c in range(NDT // 2):
                for pc in range(NPC):
                    nc.tensor.matmul(
                        tk[:, pc * 512:(pc + 1) * 512],
                        lhsT=wk_a[:, 2 * c:2 * c + 2, jt * P:(jt + 1) * P],
                        rhs=xk_a[:, 2 * c:2 * c + 2,
                                 pc * 512:(pc + 1) * 512],
                        start=(c == 0), stop=(c == NDT // 2 - 1),
                        perf_mode=DR)
                    nc.tensor.matmul(
                        tq[:, pc * 512:(pc + 1) * 512],
                        lhsT=wq_a[:, 2 * c:2 * c + 2, jt * P:(jt + 1) * P],
                        rhs=xq_a[:, 2 * c:2 * c + 2,
                                 pc * 512:(pc + 1) * 512],
                        start=(c == 0), stop=(c == NDT // 2 - 1),
                        perf_mode=DR)
            for pc in range(NPC):
                nc.scalar.activation(
                    kpT[:, jt, pc * 512:(pc + 1) * 512],
                    tk[:, pc * 512:(pc + 1) * 512], AF.Relu,
                    bias=bkT[:, jt:jt + 1])
                nc.scalar.activation(
                    qpT[:, jt, pc * 512:(pc + 1) * 512],
                    tq[:, pc * 512:(pc + 1) * 512], AF.Relu,
                    bias=bqT[:, jt:jt + 1])

        for jt in range(NJT):
            qk_wave(jt)

        # --- V projection, natural layout -> vpa [128, st, 512] bf16
        # chains on psB tiles (1 bank each); emitted after the first two
        # scores blocks so it fills PE gaps without starving the exp stream.
        vpa = vpool.tile([P, NST, DG], BF16, tag="vpa")

        def emit_vproj():
            for st in range(NST):
                ps = psB.tile([P, 512], F32, tag="nt")
                for c in range(NDT // 2):
                    nc.tensor.matmul(
                        ps,
                        lhsT=xv_a[:, 2 * c:2 * c + 2, st * P:(st + 1) * P],
                        rhs=wv_a[:, 2 * c:2 * c + 2, :],
                        start=(c == 0), stop=False,
                        perf_mode=DR)
                nc.tensor.matmul(ps, lhsT=onesrow, rhs=bv_sb,
                                 start=False, stop=True)
                nc.vector.tensor_scalar(out=vpa[:, st, :], in0=ps,
                                        scalar1=0.0, scalar2=None, op0=ALU.max)

        # --- attention, software-pipelined across 8 (pc, hp) blocks.
        aoT3 = aopool.tile([P, NJT, S], BF16, tag="aoT3")
        blocks = [(pc, hp) for pc in range(NPC) for hp in range(NJT)]
        ex_tiles = {}

        def emit_scores(i):
            pc, hp = blocks[i]
            pslice = slice(pc * 512, (pc + 1) * 512)
            ex = epool.tile([P, NST, 1024], BF16, tag="exp")
            ex_tiles[i] = ex
            for ut in range(NST):
                uslice = slice(ut * P, (ut + 1) * P)
                pw = psA.tile([P, 1024], F32, tag="ps")
                nc.tensor.matmul(
                    pw[:, 0:512],
                    lhsT=kpT[0:DH, hp, uslice],
                    rhs=qpT[0:DH, hp, pslice],
                    start=True, stop=True)
                nc.tensor.matmul(
                    pw[:, 512:1024],
                    lhsT=kpT[DH:P, hp, uslice],
                    rhs=qpT[DH:P, hp, pslice],
                    start=True, stop=True)
                nc.scalar.activation(ex[:, ut, :], pw, AF.Exp, scale=0.125)

        def emit_attn_finish(i):
            pc, hp = blocks[i]
            pslice = slice(pc * 512, (pc + 1) * 512)
            hA, hB = 2 * hp, 2 * hp + 1
            ex = ex_tiles.pop(i)
            # attn @ v: column-paired accumulation over key tiles
            nt = psB.tile([P, 512], F32, tag="nt")
            for ut in range(NST):
                nc.tensor.matmul(
                    nt[0:DH, :],
                    lhsT=vpa[:, ut, hA * DH:(hA + 1) * DH],
                    rhs=ex[:, ut, 0:512],
                    start=(ut == 0), stop=(ut == NST - 1),
                    skip_group_check=True)
                nc.tensor.matmul(
                    nt[DH:P, :],
                    lhsT=vpa[:, ut, hB * DH:(hB + 1) * DH],
                    rhs=ex[:, ut, 512:1024],
                    start=(ut == 0), stop=(ut == NST - 1),
                    skip_group_check=True)
            # softmax denominator: DVE tree-sum over ut, two K=128 matmuls
            # with a ones column reduce partitions -> Z_A (row 0), Z_B (row
            # 32); one [33,512] ACT copy stages both (garbage rows masked by
            # the broadcast matmul).
            t1 = t1pool.tile([P, 4, 1024], BF16, tag="t1")
            nc.vector.tensor_add(t1, ex[:, 0:4, :], ex[:, 4:8, :])
            nc.vector.tensor_add(t1[:, 0:2, :], t1[:, 0:2, :], t1[:, 2:4, :])
            exsum = espool.tile([P, 1024], BF16, tag="exsum")
            nc.vector.tensor_add(exsum, t1[:, 0, :], t1[:, 1, :])
            zps = psZD.tile([P, 512], F32, tag="po")
            nc.tensor.matmul(zps[0:1, :], lhsT=onescol,
                             rhs=exsum[:, 0:512], start=True, stop=True)
            nc.tensor.matmul(zps[32:33, :], lhsT=onescol,
                             rhs=exsum[:, 512:1024], start=True, stop=True)
            zsb = zsbpool.tile([33, 512], BF16, tag="zsb")
            nc.scalar.copy(zsb, zps[0:33, :])
            # broadcast: rows 0:64 <- Z_A, rows 64:128 <- Z_B
            zbc = psZD.tile([P, 512], F32, tag="po")
            nc.tensor.matmul(zbc, lhsT=bcmask, rhs=zsb,
                             start=True, stop=True)
            rcp = rpool.tile([P, 512], F32, tag="rcp")
            nc.vector.reciprocal_approx_fast(rcp, zbc)
            nc.vector.tensor_mul(aoT3[:, hp, pslice], nt, rcp)

        def emit_outproj(pc):
            for pt in range(pc * 4, pc * 4 + 4):
                os_ = outpool.tile([P, 1024], BF16, tag="os")
                for jj in range(2):
                    po_ = psZD.tile([P, 512], F32, tag="po")
                    for hp in range(NJT):
                        nc.tensor.matmul(
                            po_,
                            lhsT=aoT3[:, hp, pt * P:(pt + 1) * P],
                            rhs=wo3[:, hp, jj * 512:(jj + 1) * 512],
                            start=(hp == 0), stop=(hp == NJT - 1))
                    nc.vector.tensor_copy(os_[:, jj * 512:(jj + 1) * 512], po_)
                nc.sync.dma_start(out=out[pt * P:(pt + 1) * P, :], in_=os_)

        # scores run two blocks ahead of attn@v (ex triple-buffered); the V
        # projection is emitted after the first two scores blocks so the exp
        # stream outranks it on PE.
        for i in range(len(blocks)):
            emit_scores(i)
            if i == 1:
                emit_vproj()
            if i >= 2:
                emit_attn_finish(i - 2)
            if i - 2 == 3:
                emit_outproj(0)
        emit_attn_finish(len(blocks) - 2)
        emit_attn_finish(len(blocks) - 1)
        emit_outproj(1)

    nc.compile()
    return nc


_CACHE = {}


def get_nc():
    if "nc" not in _CACHE:
        _CACHE["nc"] = build_bass()
    return _CACHE["nc"]


def make_in_maps(q, k, v, Wq, bq, Wk, bk, Wv, bv, Wo, bo):
    q = np.asarray(q, np.float32)
    k = np.asarray(k, np.float32)
    v = np.asarray(v, np.float32)
    Wq = np.asarray(Wq, np.float32)
    Wk = np.asarray(Wk, np.float32)
    Wv = np.asarray(Wv, np.float32)
    Wo = np.asarray(Wo, np.float32)
    bq = np.asarray(bq, np.float32)
    bk = np.asarray(bk, np.float32)
    bv = np.asarray(bv, np.float32)

    qT = [np.ascontiguousarray(q[b].T).astype(NPFP8) for b in range(B)]
    kT = [np.ascontiguousarray(k[b].T).astype(NPFP8) for b in range(B)]
    vT = [np.ascontiguousarray(v[b].T).astype(NPFP8) for b in range(B)]

    in_maps = []
    for c in range(NCORES):
        b, g = divmod(c, 2)
        sl = slice(g * DG, (g + 1) * DG)
        in_maps.append({
            "xqT": qT[b],
            "xkT": kT[b],
            "xvT": vT[b],
            "wq": np.ascontiguousarray(Wq[:, sl]).astype(NPFP8),
            "wk": np.ascontiguousarray(Wk[:, sl]).astype(NPFP8),
            "wv": np.ascontiguousarray(Wv[:, sl]).astype(NPFP8),
            "bq": np.ascontiguousarray(bq[sl]).reshape(1, DG),
            "bk": np.ascontiguousarray(bk[sl]).reshape(1, DG),
            "bv": np.ascontiguousarray(bv[sl]).reshape(1, DG).astype(NPBF16),
            "wo": np.ascontiguousarray(Wo[sl, :]).astype(NPBF16),
        })
    return in_maps


def combine_outputs(parts, bo):
    bo = np.asarray(bo, np.float32)
    out = np.empty((B, S, D), np.float32)
    for b in range(B):
        p0 = np.asarray(parts[2 * b], np.float32)
        p1 = np.asarray(parts[2 * b + 1], np.float32)
        out[b] = np.maximum(p0 + p1 + bo[None, :], 0.0)
    return out


def run(in_maps, trace=False, **kwargs):
    from concourse.bass_utils import run_bass_kernel_spmd
    nc = get_nc()
    return run_bass_kernel_spmd(nc, in_maps, list(range(NCORES)),
                                trace=trace, **kwargs)


def kernel(q, k, v, Wq, bq, Wk, bk, Wv, bv, Wo, bo):
    in_maps = make_in_maps(q, k, v, Wq, bq, Wk, bk, Wv, bv, Wo, bo)
    res = run(in_maps)
    parts = [res.results[c]["out"] for c in range(NCORES)]
    return combine_outputs(parts, bo)
